# revision 28
# baseline (speedup 1.0000x reference)
"""Causal self-attention (B=4, T=2048, HID=2048, NH=16, HD=128) on 8 TRN2 cores.

Tensor-parallel over heads (2 heads/core). v8: keep the PE stream
continuous (stalls also drop the PE out of its max p-state, so every
bubble costs ~2x). den stays on the PE as per-k ones^T@slab accumulation
(v4's Pool-chain den stalled the in-order PE queue for ~150us). vs v3:
  - No Ln on ScalarE (was 33 ACT_TABLE_LOADs = 42us): rstd = rsqrt(ssq)
    seeded as exp(a*bits(ssq)+b) (Mitchell log via the fp32 bit pattern,
    Exp stays table-resident) + one DVE Newton step.
  - k normalized on DVE (rstd_k folded into kT), so the slab exp has
    constant scale/bias and no cross-engine scale-AP dependency.
  - Host-side x1|x2 column permutation of Wq/Wk: rope/rmsnorm run on
    contiguous DVE views; cos/sin pre-tiled to 256 cols.
  - Transposes drain one sub-block at a time through a queue popped
    between qkv sub-blocks and attn k-iters: the psum ring never forces
    a PE wait on the scalar evacuation copy.
  - PV AND den both delayed by TWO k-blocks so exp+mask latency never
    stalls the PE queue.
  - y psum drains on vector so scalar's exp stream paces attn unimpeded.
  - mask+wo (8.5MB) DMA triggers issue after group 1 so the xt prefetch
    stream is never queued behind them (quartering wo into strided 2MB
    slices instead REGRESSED 130us - keep whole-tile transfers).
  - First x tile loaded as four independent per-sub tiles and wqkv in
    four quarters so the first matmul waits on 0.5MB, not 5MB.
  - rope tables 64-wide with stride-0 broadcast views (SBUF headroom).
(v4's Pool-accumulated den and v6's deferred ones^T@S both LOST time:
attn is exp-paced, so den matmuls on the PE are free bubble-fill, and
any cross-engine dependency entering the in-order PE queue stalls it
and drops the PE p-state. fp8/DoubleRow is numerically dead here: every
quantization site alone measures 1.3-5e-2 max-rel vs the 2e-2 gate.)
"""

import sys

if "/opt/trn_rl_repo" not in sys.path:
    sys.path.insert(0, "/opt/trn_rl_repo")

from contextlib import ExitStack

import numpy as np

import concourse.bass as bass
import concourse.tile as tile
from concourse import bacc, mybir
from concourse.bass_utils import run_bass_kernel_spmd

F32 = mybir.dt.float32
F16 = mybir.dt.float16
I32 = mybir.dt.int32
AF = mybir.ActivationFunctionType
ALU = mybir.AluOpType
AX = mybir.AxisListType

B, T, HID = 4, 2048, 2048
NH, HD = 16, 128
N_CORES = 8
NHC = NH // N_CORES          # heads per core = 2
NC = NHC * HD                # per-core head cols = 256
TM = B * T
TBB = T // 128               # 16 t-blocks per batch
KC = HID // 128              # 16 contraction chunks
ROPE_BASE = 10000.0
EXP_BIAS = -1.25

# Mitchell rsqrt seed: rsqrt(x) ~= exp(A*float(bits(x)) + Bc)
LN2 = float(np.log(2.0))
MITCH_SIGMA = 0.0430357
RSQ_A = -0.5 * LN2 / (1 << 23)
RSQ_B = 0.5 * LN2 * (127.0 + MITCH_SIGMA)


def build_program():
    nc = bacc.Bacc("TRN2", target_bir_lowering=False, debug=False,
                   num_devices=N_CORES)

    xT = nc.dram_tensor("xT", [HID, TM], F16, kind="ExternalInput").ap()
    wqkvd = nc.dram_tensor("wqkv", [HID, 3 * NC], F16,
                           kind="ExternalInput").ap()
    wod = nc.dram_tensor("wo", [NC, HID], F16, kind="ExternalInput").ap()
    cosd = nc.dram_tensor("cos", [T, 64], F16, kind="ExternalInput").ap()
    sind = nc.dram_tensor("sin", [T, 64], F16, kind="ExternalInput").ap()
    w2d = nc.dram_tensor("w2", [128, 256], F16, kind="ExternalInput").ap()
    maskd = nc.dram_tensor("masks", [4, 128, 512], F16, kind="ExternalInput").ap()
    identd = nc.dram_tensor("ident", [128, 128], F16, kind="ExternalInput").ap()
    y = nc.dram_tensor("y", [HID, TM], F16, kind="ExternalOutput").ap()

    with tile.TileContext(nc) as tc, ExitStack() as ctx:
        consts = ctx.enter_context(tc.tile_pool(name="consts", bufs=1))
        xt_pool = ctx.enter_context(tc.tile_pool(name="xt", bufs=2))

        # first x tile as four separate per-sub tiles so the first matmul
        # depends only on the first 0.5MB transfer, not all of xt0
        xt0_src = xT[:, 0:512].rearrange("(k1 k2) t -> k2 k1 t", k2=128)
        xt0 = []
        for s in range(4):
            t0s = xt_pool.tile([128, KC, 128], F16, tag=f"xt0{s}",
                               name=f"xt00_{s}", bufs=1)
            xt0.append(t0s)
        nc.sync.dma_start(out=xt0[0], in_=xt0_src[:, :, bass.ts(0, 128)])
        # weight upload in 4 quarters so accumulation chunk k unblocks early
        wqkv_r = wqkvd.rearrange("(k1 k2) n -> k2 k1 n", k2=128)
        wq_sb = []
        for qtr in range(4):
            wt = consts.tile([128, KC // 4, 3 * NC], F16, tag=f"wqkv{qtr}",
                             name=f"wqkv{qtr}")
            nc.sync.dma_start(out=wt, in_=wqkv_r[:, qtr * 4:(qtr + 1) * 4, :])
            wq_sb.append(wt)
        for s in range(1, 4):
            nc.sync.dma_start(out=xt0[s], in_=xt0_src[:, :, bass.ts(s, 128)])
        # everything else is first needed >=18us in; triggers are issued
        # after the first qkv group (the sync engine fires in program order)
        ident = consts.tile([128, 128], F16, tag="ident")
        cos_sb = consts.tile([128, TBB, 64], F16, tag="cos")
        sin_sb = consts.tile([128, TBB, 64], F16, tag="sin")
        w2_sb = consts.tile([128, 256], F16, tag="w2")
        mask_sb = consts.tile([128, 4, 512], F16, tag="mask")
        wo_sb = consts.tile([128, NHC, HID], F16, tag="wo")

        def load_mid_consts():
            # needed by group-0's rope/norm and the first transposes
            nc.sync.dma_start(out=cos_sb,
                              in_=cosd.rearrange("(t1 t2) j -> t2 t1 j", t2=128))
            nc.sync.dma_start(out=sin_sb,
                              in_=sind.rearrange("(t1 t2) j -> t2 t1 j", t2=128))
            nc.sync.dma_start(out=w2_sb, in_=w2d)
            nc.sync.dma_start(out=ident, in_=identd)

        def load_late_consts():
            nc.sync.dma_start(out=mask_sb,
                              in_=maskd.rearrange("m p t -> p m t"))
            nc.sync.dma_start(
                out=wo_sb, in_=wod.rearrange("(n1 n2) c -> n2 n1 c", n2=128))
        ones_bc = consts.tile([128, 128], F16, tag="ones")
        nc.vector.memset(ones_bc, 1.0)
        negc = consts.tile([128, 1], F32, tag="negc")
        nc.vector.memset(negc, EXP_BIAS)
        rsqb = consts.tile([128, 1], F32, tag="rsqb")
        nc.vector.memset(rsqb, RSQ_B)

        # PSUM banks (8): qk*2 + (st|v shared, phase-disjoint)*2 + acc*2
        # (outT+den ring) + tr*1 + y*1
        ps_qk = ctx.enter_context(tc.tile_pool(name="ps_qk", bufs=2, space="PSUM"))
        ps_sv = ctx.enter_context(tc.tile_pool(name="ps_sv", bufs=2, space="PSUM"))
        ps_tr = ctx.enter_context(tc.tile_pool(name="ps_tr", bufs=1, space="PSUM"))
        ps_acc = ctx.enter_context(tc.tile_pool(name="ps_acc", bufs=2, space="PSUM"))
        ps_y = ctx.enter_context(tc.tile_pool(name="ps_y", bufs=1, space="PSUM"))

        res = ctx.enter_context(tc.tile_pool(name="res", bufs=1))
        g_pool = ctx.enter_context(tc.tile_pool(name="gp", bufs=2))
        slab_pool = ctx.enter_context(tc.tile_pool(name="slab", bufs=2))
        rec_pool = ctx.enter_context(tc.tile_pool(name="rc", bufs=2))
        y_pool = ctx.enter_context(tc.tile_pool(name="yo", bufs=3))

        proj_jobs = []
        tr_jobs = []

        drain_mode = [False]

        def emit_proj_job(job):
            bb, cb, tg, aT = job
            if drain_mode[0]:
                # post-loop: attn psum banks are dead; double-buffer there
                y_ps = ps_acc.tile([128, 512], F32, tag="acc",
                                   name=f"yps{bb}_{cb}_{tg}")
            else:
                y_ps = ps_y.tile([128, 512], F32, tag="yacc",
                                 name=f"yps{bb}_{cb}_{tg}")
            for n in range(NHC):
                nc.tensor.matmul(y_ps, wo_sb[:, n, bass.ts(cb, 128)],
                                 aT[:, n, bass.ds(tg * 512, 512)],
                                 start=(n == 0), stop=(n == NHC - 1))
            ysb = y_pool.tile([128, 512], F16, tag="ysb",
                              name=f"ysb{bb}_{cb}_{tg}")
            # y drains on vector (scalar's exp stream paces the attn
            # phase); the post-loop drain alternates both idle engines
            if drain_mode[0] and cb % 2 == 0:
                nc.scalar.copy(ysb, y_ps)
            else:
                nc.vector.tensor_copy(ysb, y_ps)
            nc.sync.dma_start(
                out=y[bass.ts(cb, 128), bass.ds(bb * T + tg * 512, 512)],
                in_=ysb)

        def emit_tr_sub(job):
            nrmq, nrmk, g, sub, qkT = job
            tbl = 4 * g + sub
            t_ps = ps_tr.tile([128, 4, 128], F16, tag="tr",
                              name=f"tps{tbl}")
            for h in range(2):
                nc.tensor.transpose(
                    t_ps[:, h, :],
                    nrmq[:, sub, h, :, :].rearrange("p x d -> p (x d)"),
                    ident)
                nc.tensor.transpose(
                    t_ps[:, 2 + h, :],
                    nrmk[:, sub, h, :, :].rearrange("p x d -> p (x d)"),
                    ident)
            nc.scalar.copy(qkT[:, :, bass.ds(tbl * 128, 128)], t_ps)

        def qkv_group(b, g, qkT, v_t, xt_pre=None, post_subs=None):
            if xt_pre is not None:
                xt = None
            else:
                xt = xt_pool.tile([128, KC, 512], F16, tag="xt")
                nc.sync.dma_start(
                    out=xt,
                    in_=xT[:, bass.ds((b * TBB + 4 * g) * 128, 512)]
                    .rearrange("(k1 k2) t -> k2 k1 t", k2=128))
            qk16 = g_pool.tile([128, 4, 512], F16, tag="qk16")
            for sub in range(4):
                tbl = 4 * g + sub
                qk_ps = ps_qk.tile([128, 512], F32, tag="qk",
                                   name=f"qkv{b}_{g}_{sub}")
                v_ps = ps_sv.tile([128, 256], F32, tag="sv",
                                  name=f"v{b}_{g}_{sub}")
                for k1 in range(KC):
                    lhs = (xt_pre[sub][:, k1, :] if xt_pre is not None
                           else xt[:, k1, bass.ts(sub, 128)])
                    st, sp = (k1 == 0), (k1 == KC - 1)
                    wsb = wq_sb[k1 // 4]
                    kk = k1 % 4
                    nc.tensor.matmul(qk_ps, lhs,
                                     wsb[:, kk, 0:512], start=st, stop=sp)
                    nc.tensor.matmul(v_ps, lhs,
                                     wsb[:, kk, 512:768], start=st, stop=sp)
                nc.scalar.copy(qk16[:, sub, :], qk_ps)
                nc.scalar.copy(v_t[:, tbl, :], v_ps)
                if tr_jobs:
                    emit_tr_sub(tr_jobs.pop(0))
            if post_subs is not None:
                post_subs()

            # rope on all 4 t-blocks, contiguous x1|x2 halves
            rot = g_pool.tile([128, 4, 512], F16, tag="rot")
            tmp = g_pool.tile([128, 4, 256], F16, tag="tmp")
            x1, x2 = qk16[:, :, 0:256], qk16[:, :, 256:512]
            r1, r2 = rot[:, :, 0:256], rot[:, :, 256:512]
            ct = (cos_sb[:, 4 * g:4 * g + 4, None, :]
                  .broadcast_to([128, 4, 4, 64]))
            sn = (sin_sb[:, 4 * g:4 * g + 4, None, :]
                  .broadcast_to([128, 4, 4, 64]))
            r1v = r1.rearrange("p t (s d) -> p t s d", s=4)
            r2v = r2.rearrange("p t (s d) -> p t s d", s=4)
            x1v = x1.rearrange("p t (s d) -> p t s d", s=4)
            x2v = x2.rearrange("p t (s d) -> p t s d", s=4)
            tmpv = tmp.rearrange("p t (s d) -> p t s d", s=4)
            nc.vector.tensor_mul(r1v, x1v, ct)
            nc.vector.tensor_mul(tmpv, x2v, sn)
            nc.vector.tensor_sub(r1, r1, tmp)
            nc.vector.tensor_mul(r2v, x2v, ct)
            nc.vector.tensor_mul(tmpv, x1v, sn)
            nc.vector.tensor_add(r2, r2, tmp)

            # ssq per (t-block, slot): slots = (qh0, qh1, kh0, kh1)
            sq = g_pool.tile([128, 4, 512], F16, tag="sq")
            nc.vector.tensor_mul(sq, rot, rot)
            ssq2 = g_pool.tile([128, 4, 2, 4], F32, tag="ssq2")
            nc.vector.tensor_reduce(
                ssq2.rearrange("p t x s -> p (t x s)"),
                sq.rearrange("p t (x s d) -> p (t x s) d", x=2, s=4),
                axis=AX.X, op=ALU.add)
            ssq = g_pool.tile([128, 4, 4], F32, tag="ssq")
            nc.vector.tensor_add(ssq, ssq2[:, :, 0, :], ssq2[:, :, 1, :])
            # rstd = rsqrt(ssq): Mitchell exp seed + one Newton step, no Ln
            fbits = g_pool.tile([128, 4, 4], F32, tag="fbits")
            nc.vector.tensor_copy(fbits, ssq.bitcast(I32))
            y0 = g_pool.tile([128, 4, 4], F32, tag="y0")
            nc.scalar.activation(y0, fbits, AF.Exp, scale=RSQ_A, bias=rsqb)
            t1 = g_pool.tile([128, 4, 4], F32, tag="t1")
            nc.vector.tensor_mul(t1, y0, y0)
            nc.vector.tensor_mul(t1, t1, ssq)
            nc.vector.tensor_scalar(t1, t1, -0.5, 1.5, ALU.mult, ALU.add)
            rstd = g_pool.tile([128, 4, 4], F32, tag="rstd")
            nc.vector.tensor_mul(rstd, y0, t1)

            # q_hat = rope(q) * w2 * rstd_q ; k_hat = rope(k) * rstd_k
            # layout [t, head, half, d]: each head a contiguous 128-run for
            # the transposes; DVE caps at 3 free dims so work per half
            nrmq = g_pool.tile([128, 4, 2, 2, 64], F16, tag="nrmq")
            nrmk = g_pool.tile([128, 4, 2, 2, 64], F16, tag="nrmk")
            for xh in range(2):
                rq = (rot[:, :, bass.ds(xh * 256, 128)]
                      .rearrange("p t (s d) -> p t s d", s=2))
                w2h = (w2_sb[:, bass.ds(xh * 128, 128)]
                       .rearrange("p (s d) -> p s d", s=2))
                nc.vector.tensor_mul(
                    nrmq[:, :, :, xh, :], rq,
                    w2h[:, None].broadcast_to([128, 4, 2, 64]))
                rk = (rot[:, :, bass.ds(xh * 256 + 128, 128)]
                      .rearrange("p t (s d) -> p t s d", s=2))
                nc.vector.tensor_mul(
                    nrmk[:, :, :, xh, :], rk,
                    rstd[:, :, 2:4, None].broadcast_to([128, 4, 2, 64]))
            nc.vector.tensor_mul(
                nrmq.rearrange("p t s x d -> p t s (x d)"),
                nrmq.rearrange("p t s x d -> p t s (x d)"),
                rstd[:, :, 0:2, None].broadcast_to([128, 4, 2, 128]))
            for sub in range(4):
                tr_jobs.append((nrmq, nrmk, g, sub, qkT))

        def attn(b, j, h, qkT, v_t, aT):
            nk = 4 * j + 4
            slab = slab_pool.tile([128, TBB, 512], F16, tag="slab",
                                  name=f"slab{b}_{h}_{j}")
            outT = ps_acc.tile([128, 512], F32, tag="acc",
                               name=f"outT{b}_{h}_{j}")
            den = ps_acc.tile([128, 512], F32, tag="acc",
                              name=f"den{b}_{h}_{j}")
            qrhs = qkT[:, h, bass.ds(j * 512, 512)]
            kq = qkT[:, 2 + h, :]
            for k in range(nk):
                st_ps = ps_sv.tile([128, 512], F32, tag="sv",
                                   name=f"st{b}_{h}_{j}_{k}")
                nc.tensor.matmul(st_ps, kq[:, bass.ts(k, 128)], qrhs,
                                 start=True, stop=True)
                nc.scalar.activation(slab[:, k, :], st_ps, AF.Exp,
                                     bias=negc, scale=1.0)
                if k >= 4 * j:
                    eng = nc.vector if j == 0 else nc.gpsimd
                    eng.tensor_mul(slab[:, k, :], slab[:, k, :],
                                   mask_sb[:, k - 4 * j, :])
                if k >= 2:
                    nc.tensor.matmul(den, ones_bc, slab[:, k - 2, :],
                                     start=(k == 2), stop=False)
                    nc.tensor.matmul(outT, v_t[:, k - 2, bass.ds(h * HD, HD)],
                                     slab[:, k - 2, :],
                                     start=(k == 2), stop=False)
                if tr_jobs:
                    emit_tr_sub(tr_jobs.pop(0))
                elif proj_jobs:
                    emit_proj_job(proj_jobs.pop(0))
            nc.tensor.matmul(den, ones_bc, slab[:, nk - 2, :],
                             start=False, stop=False)
            nc.tensor.matmul(outT, v_t[:, nk - 2, bass.ds(h * HD, HD)],
                             slab[:, nk - 2, :], start=False, stop=False)
            nc.tensor.matmul(den, ones_bc, slab[:, nk - 1, :],
                             start=False, stop=True)
            nc.tensor.matmul(outT, v_t[:, nk - 1, bass.ds(h * HD, HD)],
                             slab[:, nk - 1, :], start=False, stop=True)
            rec = rec_pool.tile([128, 512], F32, tag="rec",
                                name=f"rec{b}_{h}_{j}")
            nc.vector.reciprocal_approx_fast(rec, den)
            nc.vector.tensor_mul(aT[:, h, bass.ds(j * 512, 512)], outT, rec)
            if h == NHC - 1:
                for cb in range(HID // 128):
                    proj_jobs.append((b, cb, j, aT))

        for b in range(B):
            qkT = res.tile([128, 4, T], F16, name=f"qkT{b}", tag="qkT")
            v_t = res.tile([128, TBB, NC], F16, name=f"v{b}", tag="v")
            aT = res.tile([128, NHC, T], F16, name=f"aT{b}", tag=f"aT{b % 2}")
            for g in range(TBB // 4):
                first = (b == 0 and g == 0)
                qkv_group(b, g, qkT, v_t,
                          xt_pre=xt0 if first else None,
                          post_subs=load_mid_consts if first else None)
                if b == 0 and g == 1:
                    # after group 1 so xt(g1)'s prefetch DMA is queued ahead
                    # of the 8.5MB mask+wo transfers on the sync stream
                    load_late_consts()
            # j-major so both heads' aT columns for tg=j finish early and
            # that column's proj jobs can interleave into the remaining attn
            for j in range(T // 512):
                for h in range(NHC):
                    attn(b, j, h, qkT, v_t, aT)
        while tr_jobs:
            emit_tr_sub(tr_jobs.pop(0))
        drain_mode[0] = True
        while proj_jobs:
            emit_proj_job(proj_jobs.pop(0))

    nc.compile()
    return nc


_CACHE = {}


def _get_program():
    if "nc" not in _CACHE:
        _CACHE["nc"] = build_program()
    return _CACHE["nc"]


def _host_tables():
    inv = 1.0 / (ROPE_BASE ** (np.arange(0, HD, 2, dtype=np.float32) / HD))
    freqs = np.arange(T, dtype=np.float32)[:, None] * inv[None, :]
    cos = np.cos(freqs).astype(np.float16)
    sin = np.sin(freqs).astype(np.float16)
    m = np.zeros((4, 128, 512), dtype=np.float16)
    s_idx = np.arange(128)[:, None]
    t_idx = np.arange(512)[None, :]
    for off in range(4):
        m[off] = ((off * 128 + s_idx) <= t_idx).astype(np.float16)
    return cos, sin, m


def kernel(x, Wq, Wk, Wv, Wo, q_rms_w, k_rms_w, **_):
    nc = _get_program()
    cos, sin, masks = _host_tables()
    xT = np.ascontiguousarray(
        np.asarray(x, dtype=np.float32).reshape(TM, HID).T).astype(np.float16)
    w2 = (np.asarray(q_rms_w, dtype=np.float32)
          * np.asarray(k_rms_w, dtype=np.float32) * np.sqrt(HD))
    # permuted q-col layout [half, head, d]
    w2p = np.stack([np.stack([w2[0:64]] * NHC), np.stack([w2[64:128]] * NHC)])
    w2_b = np.ascontiguousarray(
        np.broadcast_to(w2p.reshape(-1)[None, :], (128, 256))).astype(np.float16)
    ident_h = np.eye(128, dtype=np.float16)

    in_maps = []
    for c in range(N_CORES):
        cols = slice(c * NC, (c + 1) * NC)
        # qk columns permuted to [half(x1|x2), tensor(q|k), head, d]
        qk = np.stack([np.asarray(Wq)[:, cols], np.asarray(Wk)[:, cols]], 1)
        qk = qk.reshape(HID, 2, NHC, 2, 64).transpose(0, 3, 1, 2, 4)
        qk = np.ascontiguousarray(qk.reshape(HID, 2 * NC))
        in_maps.append({
            "xT": xT,
            "wqkv": np.ascontiguousarray(
                np.concatenate([qk, np.asarray(Wv)[:, cols]], axis=1)
            ).astype(np.float16),
            "wo": np.ascontiguousarray(Wo[cols, :]).astype(np.float16),
            "cos": cos, "sin": sin, "w2": w2_b, "masks": masks,
            "ident": ident_h,
        })

    res = run_bass_kernel_spmd(nc, in_maps, list(range(N_CORES)))
    out = res.results[0]["y"].astype(np.float32)
    for c in range(1, N_CORES):
        out += res.results[c]["y"]
    return np.ascontiguousarray(out.T).reshape(B, T, HID).astype(np.float32)


# revision 29
# speedup vs baseline: 1.0251x; 1.0251x over previous
"""Causal self-attention (B=4, T=2048, HID=2048, NH=16, HD=128) on 8 TRN2 cores.

Tensor-parallel over heads (2 heads/core). v8: keep the PE stream
continuous (stalls also drop the PE out of its max p-state, so every
bubble costs ~2x). den stays on the PE as per-k ones^T@slab accumulation
(v4's Pool-chain den stalled the in-order PE queue for ~150us). vs v3:
  - No Ln on ScalarE (was 33 ACT_TABLE_LOADs = 42us): rstd = rsqrt(ssq)
    seeded as exp(a*bits(ssq)+b) (Mitchell log via the fp32 bit pattern,
    Exp stays table-resident) + one DVE Newton step.
  - k normalized on DVE (rstd_k folded into kT), so the slab exp has
    constant scale/bias and no cross-engine scale-AP dependency.
  - Host-side x1|x2 column permutation of Wq/Wk: rope/rmsnorm run on
    contiguous DVE views; cos/sin pre-tiled to 256 cols.
  - Transposes drain one sub-block at a time through a queue popped
    between qkv sub-blocks and attn k-iters: the psum ring never forces
    a PE wait on the scalar evacuation copy.
  - PV AND den both delayed by TWO k-blocks so exp+mask latency never
    stalls the PE queue.
  - y psum drains on vector so scalar's exp stream paces attn unimpeded.
  - mask+wo (8.5MB) DMA triggers issue after group 1 so the xt prefetch
    stream is never queued behind them (quartering wo into strided 2MB
    slices instead REGRESSED 130us - keep whole-tile transfers).
  - First x tile loaded as four independent per-sub tiles and wqkv in
    four quarters so the first matmul waits on 0.5MB, not 5MB.
  - rope tables 64-wide with stride-0 broadcast views (SBUF headroom).
(v4's Pool-accumulated den and v6's deferred ones^T@S both LOST time:
attn is exp-paced, so den matmuls on the PE are free bubble-fill, and
any cross-engine dependency entering the in-order PE queue stalls it
and drops the PE p-state. fp8/DoubleRow is numerically dead here: every
quantization site alone measures 1.3-5e-2 max-rel vs the 2e-2 gate.)
"""

import sys

if "/opt/trn_rl_repo" not in sys.path:
    sys.path.insert(0, "/opt/trn_rl_repo")

from contextlib import ExitStack

import numpy as np

import concourse.bass as bass
import concourse.tile as tile
from concourse import bacc, mybir
from concourse.bass_utils import run_bass_kernel_spmd

F32 = mybir.dt.float32
F16 = mybir.dt.float16
I32 = mybir.dt.int32
AF = mybir.ActivationFunctionType
ALU = mybir.AluOpType
AX = mybir.AxisListType

B, T, HID = 4, 2048, 2048
NH, HD = 16, 128
N_CORES = 8
NHC = NH // N_CORES          # heads per core = 2
NC = NHC * HD                # per-core head cols = 256
TM = B * T
TBB = T // 128               # 16 t-blocks per batch
KC = HID // 128              # 16 contraction chunks
ROPE_BASE = 10000.0
EXP_BIAS = -1.25

# Mitchell rsqrt seed: rsqrt(x) ~= exp(A*float(bits(x)) + Bc)
LN2 = float(np.log(2.0))
MITCH_SIGMA = 0.0430357
RSQ_A = -0.5 * LN2 / (1 << 23)
RSQ_B = 0.5 * LN2 * (127.0 + MITCH_SIGMA)


def build_program():
    nc = bacc.Bacc("TRN2", target_bir_lowering=False, debug=False,
                   num_devices=N_CORES)

    xT = nc.dram_tensor("xT", [HID, TM], F16, kind="ExternalInput").ap()
    wqkvd = nc.dram_tensor("wqkv", [HID, 3 * NC], F16,
                           kind="ExternalInput").ap()
    wod = nc.dram_tensor("wo", [NC, HID], F16, kind="ExternalInput").ap()
    cosd = nc.dram_tensor("cos", [T, 64], F16, kind="ExternalInput").ap()
    sind = nc.dram_tensor("sin", [T, 64], F16, kind="ExternalInput").ap()
    w2d = nc.dram_tensor("w2", [128, 256], F16, kind="ExternalInput").ap()
    maskd = nc.dram_tensor("masks", [4, 128, 512], F16, kind="ExternalInput").ap()
    identd = nc.dram_tensor("ident", [128, 128], F16, kind="ExternalInput").ap()
    y = nc.dram_tensor("y", [HID, TM], F16, kind="ExternalOutput").ap()

    with tile.TileContext(nc) as tc, ExitStack() as ctx:
        consts = ctx.enter_context(tc.tile_pool(name="consts", bufs=1))
        xt_pool = ctx.enter_context(tc.tile_pool(name="xt", bufs=2))

        # first x tile as four separate per-sub tiles so the first matmul
        # depends only on the first 0.5MB transfer, not all of xt0
        xt0_src = xT[:, 0:512].rearrange("(k1 k2) t -> k2 k1 t", k2=128)
        xt0 = []
        for s in range(4):
            t0s = xt_pool.tile([128, KC, 128], F16, tag=f"xt0{s}",
                               name=f"xt00_{s}", bufs=1)
            xt0.append(t0s)
        nc.sync.dma_start(out=xt0[0], in_=xt0_src[:, :, bass.ts(0, 128)])
        # weight upload in 4 quarters so accumulation chunk k unblocks early
        wqkv_r = wqkvd.rearrange("(k1 k2) n -> k2 k1 n", k2=128)
        wq_sb = []
        for qtr in range(4):
            wt = consts.tile([128, KC // 4, 3 * NC], F16, tag=f"wqkv{qtr}",
                             name=f"wqkv{qtr}")
            nc.sync.dma_start(out=wt, in_=wqkv_r[:, qtr * 4:(qtr + 1) * 4, :])
            wq_sb.append(wt)
        for s in range(1, 4):
            nc.sync.dma_start(out=xt0[s], in_=xt0_src[:, :, bass.ts(s, 128)])
        # everything else is first needed >=18us in; triggers are issued
        # after the first qkv group (the sync engine fires in program order)
        ident = consts.tile([128, 128], F16, tag="ident")
        cos_sb = consts.tile([128, TBB, 64], F16, tag="cos")
        sin_sb = consts.tile([128, TBB, 64], F16, tag="sin")
        w2_sb = consts.tile([128, 256], F16, tag="w2")
        mask_sb = consts.tile([128, 4, 512], F16, tag="mask")
        wo_sb = consts.tile([128, NHC, HID], F16, tag="wo")

        def load_mid_consts():
            # needed by group-0's rope/norm and the first transposes
            nc.sync.dma_start(out=cos_sb,
                              in_=cosd.rearrange("(t1 t2) j -> t2 t1 j", t2=128))
            nc.sync.dma_start(out=sin_sb,
                              in_=sind.rearrange("(t1 t2) j -> t2 t1 j", t2=128))
            nc.sync.dma_start(out=w2_sb, in_=w2d)
            nc.sync.dma_start(out=ident, in_=identd)

        def load_late_consts():
            nc.sync.dma_start(out=mask_sb,
                              in_=maskd.rearrange("m p t -> p m t"))
            nc.sync.dma_start(
                out=wo_sb, in_=wod.rearrange("(n1 n2) c -> n2 n1 c", n2=128))
        ones_bc = consts.tile([128, 128], F16, tag="ones")
        nc.vector.memset(ones_bc, 1.0)
        negc = consts.tile([128, 1], F32, tag="negc")
        nc.vector.memset(negc, EXP_BIAS)
        rsqb = consts.tile([128, 1], F32, tag="rsqb")
        nc.vector.memset(rsqb, RSQ_B)

        # PSUM banks (8): qk*2 + (st|v shared, phase-disjoint)*2 + acc*2
        # (outT+den ring) + tr*1 + y*1
        ps_qk = ctx.enter_context(tc.tile_pool(name="ps_qk", bufs=2, space="PSUM"))
        ps_sv = ctx.enter_context(tc.tile_pool(name="ps_sv", bufs=2, space="PSUM"))
        ps_tr = ctx.enter_context(tc.tile_pool(name="ps_tr", bufs=1, space="PSUM"))
        ps_acc = ctx.enter_context(tc.tile_pool(name="ps_acc", bufs=2, space="PSUM"))
        ps_y = ctx.enter_context(tc.tile_pool(name="ps_y", bufs=1, space="PSUM"))

        res = ctx.enter_context(tc.tile_pool(name="res", bufs=1))
        g_pool = ctx.enter_context(tc.tile_pool(name="gp", bufs=2))
        slab_pool = ctx.enter_context(tc.tile_pool(name="slab", bufs=2))
        rec_pool = ctx.enter_context(tc.tile_pool(name="rc", bufs=2))
        y_pool = ctx.enter_context(tc.tile_pool(name="yo", bufs=3))

        proj_jobs = []
        tr_jobs = []

        drain_mode = [False]

        def emit_proj_job(job):
            bb, cb, tg, aT = job
            if drain_mode[0]:
                # post-loop: attn psum banks are dead; double-buffer there
                y_ps = ps_acc.tile([128, 512], F32, tag="acc",
                                   name=f"yps{bb}_{cb}_{tg}")
            else:
                y_ps = ps_y.tile([128, 512], F32, tag="yacc",
                                 name=f"yps{bb}_{cb}_{tg}")
            for n in range(NHC):
                nc.tensor.matmul(y_ps, wo_sb[:, n, bass.ts(cb, 128)],
                                 aT[:, n, bass.ds(tg * 512, 512)],
                                 start=(n == 0), stop=(n == NHC - 1))
            ysb = y_pool.tile([128, 512], F16, tag="ysb",
                              name=f"ysb{bb}_{cb}_{tg}")
            # y drains on vector (scalar's exp stream paces the attn
            # phase); the post-loop drain alternates both idle engines
            if drain_mode[0] and cb % 2 == 0:
                nc.scalar.copy(ysb, y_ps)
            else:
                nc.vector.tensor_copy(ysb, y_ps)
            nc.sync.dma_start(
                out=y[bass.ts(cb, 128), bass.ds(bb * T + tg * 512, 512)],
                in_=ysb)

        def emit_tr_sub(job):
            nrmq, nrmk, g, sub, qkT = job
            tbl = 4 * g + sub
            t_ps = ps_tr.tile([128, 4, 128], F16, tag="tr",
                              name=f"tps{tbl}")
            for h in range(2):
                nc.tensor.transpose(
                    t_ps[:, h, :],
                    nrmq[:, sub, h, :, :].rearrange("p x d -> p (x d)"),
                    ident)
                nc.tensor.transpose(
                    t_ps[:, 2 + h, :],
                    nrmk[:, sub, h, :, :].rearrange("p x d -> p (x d)"),
                    ident)
            nc.scalar.copy(qkT[:, :, bass.ds(tbl * 128, 128)], t_ps)

        def qkv_group(b, g, qkT, v_t, xt_pre=None, post_subs=None):
            if xt_pre is not None:
                xt = None
            else:
                xt = xt_pool.tile([128, KC, 512], F16, tag="xt")
                nc.sync.dma_start(
                    out=xt,
                    in_=xT[:, bass.ds((b * TBB + 4 * g) * 128, 512)]
                    .rearrange("(k1 k2) t -> k2 k1 t", k2=128))
            qk16 = g_pool.tile([128, 4, 512], F16, tag="qk16")
            for sub in range(4):
                tbl = 4 * g + sub
                qk_ps = ps_qk.tile([128, 512], F32, tag="qk",
                                   name=f"qkv{b}_{g}_{sub}")
                v_ps = ps_sv.tile([128, 256], F32, tag="sv",
                                  name=f"v{b}_{g}_{sub}")
                for k1 in range(KC):
                    lhs = (xt_pre[sub][:, k1, :] if xt_pre is not None
                           else xt[:, k1, bass.ts(sub, 128)])
                    st, sp = (k1 == 0), (k1 == KC - 1)
                    wsb = wq_sb[k1 // 4]
                    kk = k1 % 4
                    nc.tensor.matmul(qk_ps, lhs,
                                     wsb[:, kk, 0:512], start=st, stop=sp)
                    nc.tensor.matmul(v_ps, lhs,
                                     wsb[:, kk, 512:768], start=st, stop=sp)
                nc.scalar.copy(qk16[:, sub, :], qk_ps)
                nc.scalar.copy(v_t[:, tbl, :], v_ps)
                if tr_jobs:
                    emit_tr_sub(tr_jobs.pop(0))
            if post_subs is not None:
                post_subs()

            # rope on all 4 t-blocks, contiguous x1|x2 halves
            rot = g_pool.tile([128, 4, 512], F16, tag="rot")
            tmp = g_pool.tile([128, 4, 256], F16, tag="tmp")
            x1, x2 = qk16[:, :, 0:256], qk16[:, :, 256:512]
            r1, r2 = rot[:, :, 0:256], rot[:, :, 256:512]
            ct = (cos_sb[:, 4 * g:4 * g + 4, None, :]
                  .broadcast_to([128, 4, 4, 64]))
            sn = (sin_sb[:, 4 * g:4 * g + 4, None, :]
                  .broadcast_to([128, 4, 4, 64]))
            r1v = r1.rearrange("p t (s d) -> p t s d", s=4)
            r2v = r2.rearrange("p t (s d) -> p t s d", s=4)
            x1v = x1.rearrange("p t (s d) -> p t s d", s=4)
            x2v = x2.rearrange("p t (s d) -> p t s d", s=4)
            tmpv = tmp.rearrange("p t (s d) -> p t s d", s=4)
            nc.vector.tensor_mul(r1v, x1v, ct)
            nc.vector.tensor_mul(tmpv, x2v, sn)
            nc.vector.tensor_sub(r1, r1, tmp)
            nc.vector.tensor_mul(r2v, x2v, ct)
            nc.vector.tensor_mul(tmpv, x1v, sn)
            nc.vector.tensor_add(r2, r2, tmp)

            # ssq per (t-block, slot): slots = (qh0, qh1, kh0, kh1)
            sq = g_pool.tile([128, 4, 512], F16, tag="sq")
            nc.vector.tensor_mul(sq, rot, rot)
            ssq2 = g_pool.tile([128, 4, 2, 4], F32, tag="ssq2")
            nc.vector.tensor_reduce(
                ssq2.rearrange("p t x s -> p (t x s)"),
                sq.rearrange("p t (x s d) -> p (t x s) d", x=2, s=4),
                axis=AX.X, op=ALU.add)
            ssq = g_pool.tile([128, 4, 4], F32, tag="ssq")
            nc.vector.tensor_add(ssq, ssq2[:, :, 0, :], ssq2[:, :, 1, :])
            # rstd = rsqrt(ssq): Mitchell exp seed + one Newton step, no Ln
            fbits = g_pool.tile([128, 4, 4], F32, tag="fbits")
            nc.vector.tensor_copy(fbits, ssq.bitcast(I32))
            y0 = g_pool.tile([128, 4, 4], F32, tag="y0")
            nc.scalar.activation(y0, fbits, AF.Exp, scale=RSQ_A, bias=rsqb)
            t1 = g_pool.tile([128, 4, 4], F32, tag="t1")
            nc.vector.tensor_mul(t1, y0, y0)
            nc.vector.tensor_mul(t1, t1, ssq)
            nc.vector.tensor_scalar(t1, t1, -0.5, 1.5, ALU.mult, ALU.add)
            rstd = g_pool.tile([128, 4, 4], F32, tag="rstd")
            nc.vector.tensor_mul(rstd, y0, t1)

            # q_hat = rope(q) * w2 * rstd_q ; k_hat = rope(k) * rstd_k
            # layout [t, head, half, d]: each head a contiguous 128-run for
            # the transposes; DVE caps at 3 free dims so work per half
            nrmq = g_pool.tile([128, 4, 2, 2, 64], F16, tag="nrmq")
            nrmk = g_pool.tile([128, 4, 2, 2, 64], F16, tag="nrmk")
            for xh in range(2):
                rq = (rot[:, :, bass.ds(xh * 256, 128)]
                      .rearrange("p t (s d) -> p t s d", s=2))
                w2h = (w2_sb[:, bass.ds(xh * 128, 128)]
                       .rearrange("p (s d) -> p s d", s=2))
                nc.vector.tensor_mul(
                    nrmq[:, :, :, xh, :], rq,
                    w2h[:, None].broadcast_to([128, 4, 2, 64]))
                rk = (rot[:, :, bass.ds(xh * 256 + 128, 128)]
                      .rearrange("p t (s d) -> p t s d", s=2))
                nc.vector.tensor_mul(
                    nrmk[:, :, :, xh, :], rk,
                    rstd[:, :, 2:4, None].broadcast_to([128, 4, 2, 64]))
            nc.vector.tensor_mul(
                nrmq.rearrange("p t s x d -> p t s (x d)"),
                nrmq.rearrange("p t s x d -> p t s (x d)"),
                rstd[:, :, 0:2, None].broadcast_to([128, 4, 2, 128]))
            for sub in range(4):
                tr_jobs.append((nrmq, nrmk, g, sub, qkT))

        def attn(b, j, h, qkT, v_t, aT):
            nk = 4 * j + 4
            slab = slab_pool.tile([128, TBB, 512], F16, tag="slab",
                                  name=f"slab{b}_{h}_{j}")
            outT = ps_acc.tile([128, 512], F32, tag="acc",
                               name=f"outT{b}_{h}_{j}")
            den = ps_acc.tile([128, 512], F32, tag="acc",
                              name=f"den{b}_{h}_{j}")
            qrhs = qkT[:, h, bass.ds(j * 512, 512)]
            kq = qkT[:, 2 + h, :]
            for k in range(nk):
                st_ps = ps_sv.tile([128, 512], F32, tag="sv",
                                   name=f"st{b}_{h}_{j}_{k}")
                nc.tensor.matmul(st_ps, kq[:, bass.ts(k, 128)], qrhs,
                                 start=True, stop=True)
                nc.scalar.activation(slab[:, k, :], st_ps, AF.Exp,
                                     bias=negc, scale=1.0)
                if k >= 4 * j:
                    nc.vector.tensor_mul(slab[:, k, :], slab[:, k, :],
                                         mask_sb[:, k - 4 * j, :])
                if k >= 2:
                    nc.tensor.matmul(den, ones_bc, slab[:, k - 2, :],
                                     start=(k == 2), stop=False)
                    nc.tensor.matmul(outT, v_t[:, k - 2, bass.ds(h * HD, HD)],
                                     slab[:, k - 2, :],
                                     start=(k == 2), stop=False)
                if tr_jobs:
                    emit_tr_sub(tr_jobs.pop(0))
                elif proj_jobs:
                    emit_proj_job(proj_jobs.pop(0))
            nc.tensor.matmul(den, ones_bc, slab[:, nk - 2, :],
                             start=False, stop=False)
            nc.tensor.matmul(outT, v_t[:, nk - 2, bass.ds(h * HD, HD)],
                             slab[:, nk - 2, :], start=False, stop=False)
            nc.tensor.matmul(den, ones_bc, slab[:, nk - 1, :],
                             start=False, stop=True)
            nc.tensor.matmul(outT, v_t[:, nk - 1, bass.ds(h * HD, HD)],
                             slab[:, nk - 1, :], start=False, stop=True)
            rec = rec_pool.tile([128, 512], F32, tag="rec",
                                name=f"rec{b}_{h}_{j}")
            nc.vector.reciprocal_approx_fast(rec, den)
            nc.vector.tensor_mul(aT[:, h, bass.ds(j * 512, 512)], outT, rec)
            if h == NHC - 1:
                for cb in range(HID // 128):
                    proj_jobs.append((b, cb, j, aT))

        for b in range(B):
            qkT = res.tile([128, 4, T], F16, name=f"qkT{b}", tag="qkT")
            v_t = res.tile([128, TBB, NC], F16, name=f"v{b}", tag="v")
            aT = res.tile([128, NHC, T], F16, name=f"aT{b}", tag=f"aT{b % 2}")
            for g in range(TBB // 4):
                first = (b == 0 and g == 0)
                qkv_group(b, g, qkT, v_t,
                          xt_pre=xt0 if first else None,
                          post_subs=load_mid_consts if first else None)
                if b == 0 and g == 1:
                    # after group 1 so xt(g1)'s prefetch DMA is queued ahead
                    # of the 8.5MB mask+wo transfers on the sync stream
                    load_late_consts()
            # j-major so both heads' aT columns for tg=j finish early and
            # that column's proj jobs can interleave into the remaining attn
            for j in range(T // 512):
                for h in range(NHC):
                    attn(b, j, h, qkT, v_t, aT)
        while tr_jobs:
            emit_tr_sub(tr_jobs.pop(0))
        drain_mode[0] = True
        while proj_jobs:
            emit_proj_job(proj_jobs.pop(0))

    nc.compile()
    return nc


_CACHE = {}


def _get_program():
    if "nc" not in _CACHE:
        _CACHE["nc"] = build_program()
    return _CACHE["nc"]


def _host_tables():
    inv = 1.0 / (ROPE_BASE ** (np.arange(0, HD, 2, dtype=np.float32) / HD))
    freqs = np.arange(T, dtype=np.float32)[:, None] * inv[None, :]
    cos = np.cos(freqs).astype(np.float16)
    sin = np.sin(freqs).astype(np.float16)
    m = np.zeros((4, 128, 512), dtype=np.float16)
    s_idx = np.arange(128)[:, None]
    t_idx = np.arange(512)[None, :]
    for off in range(4):
        m[off] = ((off * 128 + s_idx) <= t_idx).astype(np.float16)
    return cos, sin, m


def kernel(x, Wq, Wk, Wv, Wo, q_rms_w, k_rms_w, **_):
    nc = _get_program()
    cos, sin, masks = _host_tables()
    xT = np.ascontiguousarray(
        np.asarray(x, dtype=np.float32).reshape(TM, HID).T).astype(np.float16)
    w2 = (np.asarray(q_rms_w, dtype=np.float32)
          * np.asarray(k_rms_w, dtype=np.float32) * np.sqrt(HD))
    # permuted q-col layout [half, head, d]
    w2p = np.stack([np.stack([w2[0:64]] * NHC), np.stack([w2[64:128]] * NHC)])
    w2_b = np.ascontiguousarray(
        np.broadcast_to(w2p.reshape(-1)[None, :], (128, 256))).astype(np.float16)
    ident_h = np.eye(128, dtype=np.float16)

    in_maps = []
    for c in range(N_CORES):
        cols = slice(c * NC, (c + 1) * NC)
        # qk columns permuted to [half(x1|x2), tensor(q|k), head, d]
        qk = np.stack([np.asarray(Wq)[:, cols], np.asarray(Wk)[:, cols]], 1)
        qk = qk.reshape(HID, 2, NHC, 2, 64).transpose(0, 3, 1, 2, 4)
        qk = np.ascontiguousarray(qk.reshape(HID, 2 * NC))
        in_maps.append({
            "xT": xT,
            "wqkv": np.ascontiguousarray(
                np.concatenate([qk, np.asarray(Wv)[:, cols]], axis=1)
            ).astype(np.float16),
            "wo": np.ascontiguousarray(Wo[cols, :]).astype(np.float16),
            "cos": cos, "sin": sin, "w2": w2_b, "masks": masks,
            "ident": ident_h,
        })

    res = run_bass_kernel_spmd(nc, in_maps, list(range(N_CORES)))
    out = res.results[0]["y"].astype(np.float32)
    for c in range(1, N_CORES):
        out += res.results[c]["y"]
    return np.ascontiguousarray(out.T).reshape(B, T, HID).astype(np.float32)


# revision 31
# speedup vs baseline: 1.0396x; 1.0141x over previous
"""Causal self-attention (B=4, T=2048, HID=2048, NH=16, HD=128) on 8 TRN2 cores.

Tensor-parallel over heads (2 heads/core). v13: keep the PE stream
continuous (stalls also drop the PE out of its max p-state, so every
bubble costs ~2x). den stays on the PE as per-k ones^T@slab accumulation
(v4's Pool-chain den stalled the in-order PE queue for ~150us). vs v3:
  - No Ln on ScalarE (was 33 ACT_TABLE_LOADs = 42us): rstd = rsqrt(ssq)
    seeded as exp(a*bits(ssq)+b) (Mitchell log via the fp32 bit pattern,
    Exp stays table-resident) + one DVE Newton step.
  - k normalized on DVE (rstd_k folded into kT), so the slab exp has
    constant scale/bias and no cross-engine scale-AP dependency.
  - Host-side x1|x2 column permutation of Wq/Wk: rope/rmsnorm run on
    contiguous DVE views; cos/sin pre-tiled to 256 cols.
  - Transposes drain one sub-block at a time through a queue popped
    between qkv sub-blocks and attn k-iters: the psum ring never forces
    a PE wait on the scalar evacuation copy.
  - PV AND den both delayed by TWO k-blocks so exp+mask latency never
    stalls the PE queue.
  - y psum drains on vector so scalar's exp stream paces attn unimpeded.
  - mask+wo (8.5MB) DMA triggers issue after group 1 so the xt prefetch
    stream is never queued behind them (quartering wo into strided 2MB
    slices instead REGRESSED 130us - keep whole-tile transfers).
  - First x tile loaded as four independent per-sub tiles and wqkv in
    four quarters so the first matmul waits on 0.5MB, not 5MB.
  - rope tables 64-wide with stride-0 broadcast views (SBUF headroom).
  - post-loop proj drain double-buffers y psum in the dead attn acc
    banks and alternates its f32->f16 copies across scalar+vector.
(v4's Pool-accumulated den and v6's deferred ones^T@S both LOST time:
attn is exp-paced, so den matmuls on the PE are free bubble-fill, and
any cross-engine dependency entering the in-order PE queue stalls it
and drops the PE p-state. fp8/DoubleRow is numerically dead here: every
quantization site alone measures 1.3-5e-2 max-rel vs the 2e-2 gate.)
"""

import sys

if "/opt/trn_rl_repo" not in sys.path:
    sys.path.insert(0, "/opt/trn_rl_repo")

from contextlib import ExitStack

import numpy as np

import concourse.bass as bass
import concourse.tile as tile
from concourse import bacc, mybir
from concourse.bass_utils import run_bass_kernel_spmd

F32 = mybir.dt.float32
F16 = mybir.dt.float16
I32 = mybir.dt.int32
AF = mybir.ActivationFunctionType
ALU = mybir.AluOpType
AX = mybir.AxisListType

B, T, HID = 4, 2048, 2048
NH, HD = 16, 128
N_CORES = 8
NHC = NH // N_CORES          # heads per core = 2
NC = NHC * HD                # per-core head cols = 256
TM = B * T
TBB = T // 128               # 16 t-blocks per batch
KC = HID // 128              # 16 contraction chunks
ROPE_BASE = 10000.0
EXP_BIAS = -1.25

# Mitchell rsqrt seed: rsqrt(x) ~= exp(A*float(bits(x)) + Bc)
LN2 = float(np.log(2.0))
MITCH_SIGMA = 0.0430357
RSQ_A = -0.5 * LN2 / (1 << 23)
RSQ_B = 0.5 * LN2 * (127.0 + MITCH_SIGMA)


def build_program():
    nc = bacc.Bacc("TRN2", target_bir_lowering=False, debug=False,
                   num_devices=N_CORES)

    xT = nc.dram_tensor("xT", [HID, TM], F16, kind="ExternalInput").ap()
    wqkvd = nc.dram_tensor("wqkv", [HID, 3 * NC], F16,
                           kind="ExternalInput").ap()
    wod = nc.dram_tensor("wo", [NC, HID], F16, kind="ExternalInput").ap()
    cosd = nc.dram_tensor("cos", [T, 64], F16, kind="ExternalInput").ap()
    sind = nc.dram_tensor("sin", [T, 64], F16, kind="ExternalInput").ap()
    w2d = nc.dram_tensor("w2", [128, 256], F16, kind="ExternalInput").ap()
    maskd = nc.dram_tensor("masks", [4, 128, 512], F16, kind="ExternalInput").ap()
    identd = nc.dram_tensor("ident", [128, 128], F16, kind="ExternalInput").ap()
    y = nc.dram_tensor("y", [HID, TM], F16, kind="ExternalOutput").ap()

    with tile.TileContext(nc) as tc, ExitStack() as ctx:
        consts = ctx.enter_context(tc.tile_pool(name="consts", bufs=1))
        xt_pool = ctx.enter_context(tc.tile_pool(name="xt", bufs=2))

        # first x tile split by k1-QUARTER (keeps 1KB contiguous runs per
        # partition; the old per-sub split degraded runs to 256B),
        # interleaved with the weight quarters so both first-quarters
        # land before the first matmul needs them
        xt0_src = xT[:, 0:512].rearrange("(k1 k2) t -> k2 k1 t", k2=128)
        xt0 = xt_pool.tile([128, KC, 512], F16, tag="xt0", name="xt00",
                           bufs=1)
        wqkv_r = wqkvd.rearrange("(k1 k2) n -> k2 k1 n", k2=128)
        wq_sb = []
        for qtr in range(4):
            nc.sync.dma_start(
                out=xt0[:, qtr * 4:(qtr + 1) * 4, :],
                in_=xt0_src[:, qtr * 4:(qtr + 1) * 4, :])
            wt = consts.tile([128, KC // 4, 3 * NC], F16, tag=f"wqkv{qtr}",
                             name=f"wqkv{qtr}")
            nc.sync.dma_start(out=wt, in_=wqkv_r[:, qtr * 4:(qtr + 1) * 4, :])
            wq_sb.append(wt)
        # everything else is first needed >=18us in; triggers are issued
        # after the first qkv group (the sync engine fires in program order)
        ident = consts.tile([128, 128], F16, tag="ident")
        cos_sb = consts.tile([128, TBB, 64], F16, tag="cos")
        sin_sb = consts.tile([128, TBB, 64], F16, tag="sin")
        w2_sb = consts.tile([128, 256], F16, tag="w2")
        mask_sb = consts.tile([128, 4, 512], F16, tag="mask")
        wo_sb = consts.tile([128, NHC, HID], F16, tag="wo")

        def load_mid_consts():
            # needed by group-0's rope/norm and the first transposes
            nc.sync.dma_start(out=cos_sb,
                              in_=cosd.rearrange("(t1 t2) j -> t2 t1 j", t2=128))
            nc.sync.dma_start(out=sin_sb,
                              in_=sind.rearrange("(t1 t2) j -> t2 t1 j", t2=128))
            nc.sync.dma_start(out=w2_sb, in_=w2d)
            nc.sync.dma_start(out=ident, in_=identd)

        def load_late_consts():
            nc.sync.dma_start(out=mask_sb,
                              in_=maskd.rearrange("m p t -> p m t"))
            nc.sync.dma_start(
                out=wo_sb, in_=wod.rearrange("(n1 n2) c -> n2 n1 c", n2=128))
        ones_bc = consts.tile([128, 128], F16, tag="ones")
        nc.vector.memset(ones_bc, 1.0)
        negc = consts.tile([128, 1], F32, tag="negc")
        nc.vector.memset(negc, EXP_BIAS)
        rsqb = consts.tile([128, 1], F32, tag="rsqb")
        nc.vector.memset(rsqb, RSQ_B)

        # PSUM banks (8): qk*2 + (st|v shared, phase-disjoint)*2 + acc*2
        # (outT+den ring) + tr*1 + y*1
        ps_qk = ctx.enter_context(tc.tile_pool(name="ps_qk", bufs=2, space="PSUM"))
        ps_sv = ctx.enter_context(tc.tile_pool(name="ps_sv", bufs=2, space="PSUM"))
        ps_tr = ctx.enter_context(tc.tile_pool(name="ps_tr", bufs=1, space="PSUM"))
        ps_acc = ctx.enter_context(tc.tile_pool(name="ps_acc", bufs=2, space="PSUM"))
        ps_y = ctx.enter_context(tc.tile_pool(name="ps_y", bufs=1, space="PSUM"))

        res = ctx.enter_context(tc.tile_pool(name="res", bufs=1))
        g_pool = ctx.enter_context(tc.tile_pool(name="gp", bufs=2))
        slab_pool = ctx.enter_context(tc.tile_pool(name="slab", bufs=2))
        rec_pool = ctx.enter_context(tc.tile_pool(name="rc", bufs=2))
        y_pool = ctx.enter_context(tc.tile_pool(name="yo", bufs=3))

        proj_jobs = []
        tr_jobs = []

        drain_mode = [False]

        def emit_proj_job(job):
            bb, cb, tg, aT = job
            if drain_mode[0]:
                # post-loop: attn psum banks are dead; double-buffer there
                y_ps = ps_acc.tile([128, 512], F32, tag="acc",
                                   name=f"yps{bb}_{cb}_{tg}")
            else:
                y_ps = ps_y.tile([128, 512], F32, tag="yacc",
                                 name=f"yps{bb}_{cb}_{tg}")
            for n in range(NHC):
                nc.tensor.matmul(y_ps, wo_sb[:, n, bass.ts(cb, 128)],
                                 aT[:, n, bass.ds(tg * 512, 512)],
                                 start=(n == 0), stop=(n == NHC - 1))
            ysb = y_pool.tile([128, 512], F16, tag="ysb",
                              name=f"ysb{bb}_{cb}_{tg}")
            # y drains on vector (scalar's exp stream paces the attn
            # phase); the post-loop drain alternates both idle engines
            if drain_mode[0] and cb % 2 == 0:
                nc.scalar.copy(ysb, y_ps)
            else:
                nc.vector.tensor_copy(ysb, y_ps)
            nc.sync.dma_start(
                out=y[bass.ts(cb, 128), bass.ds(bb * T + tg * 512, 512)],
                in_=ysb)

        def emit_tr_sub(job):
            nrmq, nrmk, g, sub, qkT = job
            tbl = 4 * g + sub
            t_ps = ps_tr.tile([128, 4, 128], F16, tag="tr",
                              name=f"tps{tbl}")
            for h in range(2):
                nc.tensor.transpose(
                    t_ps[:, h, :],
                    nrmq[:, sub, h, :, :].rearrange("p x d -> p (x d)"),
                    ident)
                nc.tensor.transpose(
                    t_ps[:, 2 + h, :],
                    nrmk[:, sub, h, :, :].rearrange("p x d -> p (x d)"),
                    ident)
            nc.scalar.copy(qkT[:, :, bass.ds(tbl * 128, 128)], t_ps)

        def qkv_group(b, g, qkT, v_t, xt_pre=None, post_subs=None):
            if xt_pre is not None:
                xt = xt_pre
            else:
                xt = xt_pool.tile([128, KC, 512], F16, tag="xt")
                nc.sync.dma_start(
                    out=xt,
                    in_=xT[:, bass.ds((b * TBB + 4 * g) * 128, 512)]
                    .rearrange("(k1 k2) t -> k2 k1 t", k2=128))
            qk16 = g_pool.tile([128, 4, 512], F16, tag="qk16")
            for sub in range(4):
                tbl = 4 * g + sub
                qk_ps = ps_qk.tile([128, 512], F32, tag="qk",
                                   name=f"qkv{b}_{g}_{sub}")
                v_ps = ps_sv.tile([128, 256], F32, tag="sv",
                                  name=f"v{b}_{g}_{sub}")
                for k1 in range(KC):
                    lhs = xt[:, k1, bass.ts(sub, 128)]
                    st, sp = (k1 == 0), (k1 == KC - 1)
                    wsb = wq_sb[k1 // 4]
                    kk = k1 % 4
                    nc.tensor.matmul(qk_ps, lhs,
                                     wsb[:, kk, 0:512], start=st, stop=sp)
                    nc.tensor.matmul(v_ps, lhs,
                                     wsb[:, kk, 512:768], start=st, stop=sp)
                nc.scalar.copy(qk16[:, sub, :], qk_ps)
                nc.scalar.copy(v_t[:, tbl, :], v_ps)
                if tr_jobs:
                    emit_tr_sub(tr_jobs.pop(0))
            if post_subs is not None:
                post_subs()

            # rope on all 4 t-blocks, contiguous x1|x2 halves
            rot = g_pool.tile([128, 4, 512], F16, tag="rot")
            tmp = g_pool.tile([128, 4, 256], F16, tag="tmp")
            x1, x2 = qk16[:, :, 0:256], qk16[:, :, 256:512]
            r1, r2 = rot[:, :, 0:256], rot[:, :, 256:512]
            ct = (cos_sb[:, 4 * g:4 * g + 4, None, :]
                  .broadcast_to([128, 4, 4, 64]))
            sn = (sin_sb[:, 4 * g:4 * g + 4, None, :]
                  .broadcast_to([128, 4, 4, 64]))
            r1v = r1.rearrange("p t (s d) -> p t s d", s=4)
            r2v = r2.rearrange("p t (s d) -> p t s d", s=4)
            x1v = x1.rearrange("p t (s d) -> p t s d", s=4)
            x2v = x2.rearrange("p t (s d) -> p t s d", s=4)
            tmpv = tmp.rearrange("p t (s d) -> p t s d", s=4)
            nc.vector.tensor_mul(r1v, x1v, ct)
            nc.vector.tensor_mul(tmpv, x2v, sn)
            nc.vector.tensor_sub(r1, r1, tmp)
            nc.vector.tensor_mul(r2v, x2v, ct)
            nc.vector.tensor_mul(tmpv, x1v, sn)
            nc.vector.tensor_add(r2, r2, tmp)

            # ssq per (t-block, slot): slots = (qh0, qh1, kh0, kh1)
            sq = g_pool.tile([128, 4, 512], F16, tag="sq")
            nc.vector.tensor_mul(sq, rot, rot)
            ssq2 = g_pool.tile([128, 4, 2, 4], F32, tag="ssq2")
            nc.vector.tensor_reduce(
                ssq2.rearrange("p t x s -> p (t x s)"),
                sq.rearrange("p t (x s d) -> p (t x s) d", x=2, s=4),
                axis=AX.X, op=ALU.add)
            ssq = g_pool.tile([128, 4, 4], F32, tag="ssq")
            nc.vector.tensor_add(ssq, ssq2[:, :, 0, :], ssq2[:, :, 1, :])
            # rstd = rsqrt(ssq): Mitchell exp seed + one Newton step, no Ln
            fbits = g_pool.tile([128, 4, 4], F32, tag="fbits")
            nc.vector.tensor_copy(fbits, ssq.bitcast(I32))
            y0 = g_pool.tile([128, 4, 4], F32, tag="y0")
            nc.scalar.activation(y0, fbits, AF.Exp, scale=RSQ_A, bias=rsqb)
            t1 = g_pool.tile([128, 4, 4], F32, tag="t1")
            nc.vector.tensor_mul(t1, y0, y0)
            nc.vector.tensor_mul(t1, t1, ssq)
            nc.vector.tensor_scalar(t1, t1, -0.5, 1.5, ALU.mult, ALU.add)
            rstd = g_pool.tile([128, 4, 4], F32, tag="rstd")
            nc.vector.tensor_mul(rstd, y0, t1)

            # q_hat = rope(q) * w2 * rstd_q ; k_hat = rope(k) * rstd_k
            # layout [t, head, half, d]: each head a contiguous 128-run for
            # the transposes; DVE caps at 3 free dims so work per half
            nrmq = g_pool.tile([128, 4, 2, 2, 64], F16, tag="nrmq")
            nrmk = g_pool.tile([128, 4, 2, 2, 64], F16, tag="nrmk")
            for xh in range(2):
                rq = (rot[:, :, bass.ds(xh * 256, 128)]
                      .rearrange("p t (s d) -> p t s d", s=2))
                w2h = (w2_sb[:, bass.ds(xh * 128, 128)]
                       .rearrange("p (s d) -> p s d", s=2))
                nc.vector.tensor_mul(
                    nrmq[:, :, :, xh, :], rq,
                    w2h[:, None].broadcast_to([128, 4, 2, 64]))
                rk = (rot[:, :, bass.ds(xh * 256 + 128, 128)]
                      .rearrange("p t (s d) -> p t s d", s=2))
                nc.vector.tensor_mul(
                    nrmk[:, :, :, xh, :], rk,
                    rstd[:, :, 2:4, None].broadcast_to([128, 4, 2, 64]))
            nc.vector.tensor_mul(
                nrmq.rearrange("p t s x d -> p t s (x d)"),
                nrmq.rearrange("p t s x d -> p t s (x d)"),
                rstd[:, :, 0:2, None].broadcast_to([128, 4, 2, 128]))
            for sub in range(4):
                tr_jobs.append((nrmq, nrmk, g, sub, qkT))

        def attn(b, j, h, qkT, v_t, aT):
            nk = 4 * j + 4
            slab = slab_pool.tile([128, TBB, 512], F16, tag="slab",
                                  name=f"slab{b}_{h}_{j}")
            outT = ps_acc.tile([128, 512], F32, tag="acc",
                               name=f"outT{b}_{h}_{j}")
            den = ps_acc.tile([128, 512], F32, tag="acc",
                              name=f"den{b}_{h}_{j}")
            qrhs = qkT[:, h, bass.ds(j * 512, 512)]
            kq = qkT[:, 2 + h, :]
            for k in range(nk):
                st_ps = ps_sv.tile([128, 512], F32, tag="sv",
                                   name=f"st{b}_{h}_{j}_{k}")
                nc.tensor.matmul(st_ps, kq[:, bass.ts(k, 128)], qrhs,
                                 start=True, stop=True)
                nc.scalar.activation(slab[:, k, :], st_ps, AF.Exp,
                                     bias=negc, scale=1.0)
                if k >= 4 * j:
                    nc.vector.tensor_mul(slab[:, k, :], slab[:, k, :],
                                         mask_sb[:, k - 4 * j, :])
                if k >= 2:
                    nc.tensor.matmul(den, ones_bc, slab[:, k - 2, :],
                                     start=(k == 2), stop=False)
                    nc.tensor.matmul(outT, v_t[:, k - 2, bass.ds(h * HD, HD)],
                                     slab[:, k - 2, :],
                                     start=(k == 2), stop=False)
                if tr_jobs:
                    emit_tr_sub(tr_jobs.pop(0))
                elif proj_jobs:
                    emit_proj_job(proj_jobs.pop(0))
            nc.tensor.matmul(den, ones_bc, slab[:, nk - 2, :],
                             start=False, stop=False)
            nc.tensor.matmul(outT, v_t[:, nk - 2, bass.ds(h * HD, HD)],
                             slab[:, nk - 2, :], start=False, stop=False)
            nc.tensor.matmul(den, ones_bc, slab[:, nk - 1, :],
                             start=False, stop=True)
            nc.tensor.matmul(outT, v_t[:, nk - 1, bass.ds(h * HD, HD)],
                             slab[:, nk - 1, :], start=False, stop=True)
            rec = rec_pool.tile([128, 512], F32, tag="rec",
                                name=f"rec{b}_{h}_{j}")
            nc.vector.reciprocal_approx_fast(rec, den)
            nc.vector.tensor_mul(aT[:, h, bass.ds(j * 512, 512)], outT, rec)
            if h == NHC - 1:
                for cb in range(HID // 128):
                    proj_jobs.append((b, cb, j, aT))

        for b in range(B):
            qkT = res.tile([128, 4, T], F16, name=f"qkT{b}", tag="qkT")
            v_t = res.tile([128, TBB, NC], F16, name=f"v{b}", tag="v")
            aT = res.tile([128, NHC, T], F16, name=f"aT{b}", tag=f"aT{b % 2}")
            for g in range(TBB // 4):
                first = (b == 0 and g == 0)
                qkv_group(b, g, qkT, v_t,
                          xt_pre=xt0 if first else None,
                          post_subs=load_mid_consts if first else None)
                if b == 0 and g == 1:
                    # after group 1 so xt(g1)'s prefetch DMA is queued ahead
                    # of the 8.5MB mask+wo transfers on the sync stream
                    load_late_consts()
            # j-major so both heads' aT columns for tg=j finish early and
            # that column's proj jobs can interleave into the remaining attn
            for j in range(T // 512):
                for h in range(NHC):
                    attn(b, j, h, qkT, v_t, aT)
        while tr_jobs:
            emit_tr_sub(tr_jobs.pop(0))
        drain_mode[0] = True
        while proj_jobs:
            emit_proj_job(proj_jobs.pop(0))

    nc.compile()
    return nc


_CACHE = {}


def _get_program():
    if "nc" not in _CACHE:
        _CACHE["nc"] = build_program()
    return _CACHE["nc"]


def _host_tables():
    inv = 1.0 / (ROPE_BASE ** (np.arange(0, HD, 2, dtype=np.float32) / HD))
    freqs = np.arange(T, dtype=np.float32)[:, None] * inv[None, :]
    cos = np.cos(freqs).astype(np.float16)
    sin = np.sin(freqs).astype(np.float16)
    m = np.zeros((4, 128, 512), dtype=np.float16)
    s_idx = np.arange(128)[:, None]
    t_idx = np.arange(512)[None, :]
    for off in range(4):
        m[off] = ((off * 128 + s_idx) <= t_idx).astype(np.float16)
    return cos, sin, m


def kernel(x, Wq, Wk, Wv, Wo, q_rms_w, k_rms_w, **_):
    nc = _get_program()
    cos, sin, masks = _host_tables()
    xT = np.ascontiguousarray(
        np.asarray(x, dtype=np.float32).reshape(TM, HID).T).astype(np.float16)
    w2 = (np.asarray(q_rms_w, dtype=np.float32)
          * np.asarray(k_rms_w, dtype=np.float32) * np.sqrt(HD))
    # permuted q-col layout [half, head, d]
    w2p = np.stack([np.stack([w2[0:64]] * NHC), np.stack([w2[64:128]] * NHC)])
    w2_b = np.ascontiguousarray(
        np.broadcast_to(w2p.reshape(-1)[None, :], (128, 256))).astype(np.float16)
    ident_h = np.eye(128, dtype=np.float16)

    in_maps = []
    for c in range(N_CORES):
        cols = slice(c * NC, (c + 1) * NC)
        # qk columns permuted to [half(x1|x2), tensor(q|k), head, d]
        qk = np.stack([np.asarray(Wq)[:, cols], np.asarray(Wk)[:, cols]], 1)
        qk = qk.reshape(HID, 2, NHC, 2, 64).transpose(0, 3, 1, 2, 4)
        qk = np.ascontiguousarray(qk.reshape(HID, 2 * NC))
        in_maps.append({
            "xT": xT,
            "wqkv": np.ascontiguousarray(
                np.concatenate([qk, np.asarray(Wv)[:, cols]], axis=1)
            ).astype(np.float16),
            "wo": np.ascontiguousarray(Wo[cols, :]).astype(np.float16),
            "cos": cos, "sin": sin, "w2": w2_b, "masks": masks,
            "ident": ident_h,
        })

    res = run_bass_kernel_spmd(nc, in_maps, list(range(N_CORES)))
    out = res.results[0]["y"].astype(np.float32)
    for c in range(1, N_CORES):
        out += res.results[c]["y"]
    return np.ascontiguousarray(out.T).reshape(B, T, HID).astype(np.float32)
